# revision 2
# baseline (speedup 1.0000x reference)
"""BiLSTM (T=256, B=64, NIN=H=NOUT=512) Trainium2 kernel over 8 NeuronCores.

Sharding: direction (2) x batch-quarter (4) = 8 cores, SPMD (one program).
Each core runs one direction's LSTM for 16 batch rows (backward cores get
time-reversed x), then computes its half of the final FC:
    out = h_f @ fc_w[:, :H].T + h_b @ fc_w[:, H:].T + fc_b
The host sums the two partial FC outputs per batch quarter. No collectives.

On-device layout (per core):
  - All matmuls "formulation G": out.T tiles [gate-rows on partitions, cols],
    lhsT = weight.T tiles (stationary), rhs = x.T / h.T tiles (moving).
  - Gate order reordered [g,i,f,o] so ACT work pipelines behind the PE.
  - h.T history kept fully resident in SBUF; xg staged through DRAM fp32.
"""

import numpy as np

T, B, NIN, H, NOUT = 256, 64, 512, 512, 512
BL = B // 4          # local batch per core (batch quarter)
KT = H // 128        # 4 k-tiles over the hidden/contraction dim
MT = (4 * H) // 128  # 16 m-tiles over the gate dim
# PyTorch gate blocks [i,f,g,o] -> our order [g,i,f,o]
GATE_PERM = [2, 0, 1, 3]

_CACHE = {}


def _build_program(t_steps):
    import concourse.mybir as mybir
    import concourse.tile as tile
    from concourse import bacc

    fp32 = mybir.dt.float32
    bf16 = mybir.dt.bfloat16
    Act = mybir.ActivationFunctionType

    ntb = t_steps * BL
    chunk = min(512, ntb)
    nch = ntb // chunk
    tpc = chunk // BL  # timesteps per xg/fc chunk

    nc = bacc.Bacc("TRN2", target_bir_lowering=False, debug=False)
    xT_d = nc.dram_tensor("xT", [128, KT, ntb], bf16, kind="ExternalInput")
    wih_d = nc.dram_tensor("wihT", [128, KT, 4 * H], bf16, kind="ExternalInput")
    whh_d = nc.dram_tensor("whhT", [128, KT, 4 * H], bf16, kind="ExternalInput")
    fcw_d = nc.dram_tensor("fcwT", [128, KT, NOUT], bf16, kind="ExternalInput")
    bias_d = nc.dram_tensor("bias", [128, MT], fp32, kind="ExternalInput")
    outT_d = nc.dram_tensor("outT", [NOUT // 128, 128, ntb], fp32, kind="ExternalOutput")

    with tile.TileContext(nc) as tc:
        with (
            tc.tile_pool(name="weights", bufs=1) as wp,
            tc.tile_pool(name="state", bufs=1) as sp,
            tc.tile_pool(name="xgdram", bufs=1, space="DRAM") as dp,
            tc.tile_pool(name="stage", bufs=3) as stp,
            tc.tile_pool(name="work", bufs=2) as wk,
            tc.tile_pool(name="cpool", bufs=2) as cp,
            tc.tile_pool(name="xgin", bufs=4) as xgp,
            tc.tile_pool(name="psg", bufs=2, space="PSUM") as psg,
            tc.tile_pool(name="psb", bufs=2, space="PSUM") as psb,
        ):
            xT = wp.tile([128, KT, ntb], bf16)
            wih = wp.tile([128, KT, 4 * H], bf16)
            whh = wp.tile([128, KT, 4 * H], bf16)
            fcw = wp.tile([128, KT, NOUT], bf16)
            bias = wp.tile([128, MT], fp32)
            h_all = sp.tile([128, KT, (t_steps + 1) * BL], bf16)
            xg_dram = dp.tile([t_steps, 128, MT * BL], fp32)

            nc.sync.dma_start(xT[:], xT_d[:])
            nc.sync.dma_start(wih[:], wih_d[:])
            nc.sync.dma_start(whh[:], whh_d[:])
            nc.sync.dma_start(fcw[:], fcw_d[:])
            nc.sync.dma_start(bias[:], bias_d[:])
            nc.vector.memset(h_all[:, :, 0:BL], 0.0)

            # ---- Phase 1: xg = W_ih @ x.T + bias, staged to DRAM ----
            # xg_dram[t, p, m*BL+b] so the per-step read is contiguous.
            for m in range(MT):
                for ch in range(nch):
                    ps = psb.tile([128, chunk], fp32, tag="big")
                    for k in range(KT):
                        nc.tensor.matmul(
                            ps[:],
                            wih[:, k, m * 128:(m + 1) * 128],
                            xT[:, k, ch * chunk:(ch + 1) * chunk],
                            start=(k == 0),
                            stop=(k == KT - 1),
                        )
                    st = stp.tile([128, chunk], fp32, tag="xgst")
                    nc.vector.tensor_scalar_add(st[:], ps[:], bias[:, m:m + 1])
                    dst = xg_dram[
                        ch * tpc:(ch + 1) * tpc, :, m * BL:(m + 1) * BL
                    ].rearrange("t p b -> p t b")
                    nc.sync.dma_start(dst, st[:].rearrange("p (t b) -> p t b", b=BL))

            # ---- Phase 2: recurrence ----
            c_prev = None
            for t in range(t_steps):
                xg_t = xgp.tile([128, MT * BL], fp32, tag="xgt")
                nc.sync.dma_start(xg_t[:], xg_dram[t, :, :])

                gp = psg.tile([128, MT * BL], fp32, tag="gates")
                u = wk.tile([128, 4 * KT * BL], fp32, tag="u")
                a = wk.tile([128, 4 * KT * BL], fp32, tag="a")
                gw = KT * BL  # 64 columns per gate group
                for j in range(4):  # gate groups in order [g, i, f, o]
                    for mm in range(4):
                        m = j * 4 + mm
                        for k in range(KT):
                            nc.tensor.matmul(
                                gp[:, m * BL:(m + 1) * BL],
                                whh[:, k, m * 128:(m + 1) * 128],
                                h_all[:, k, t * BL:(t + 1) * BL],
                                start=(k == 0),
                                stop=(k == KT - 1),
                            )
                    nc.vector.tensor_add(
                        u[:, j * gw:(j + 1) * gw],
                        gp[:, j * gw:(j + 1) * gw],
                        xg_t[:, j * gw:(j + 1) * gw],
                    )
                    nc.scalar.activation(
                        a[:, j * gw:(j + 1) * gw],
                        u[:, j * gw:(j + 1) * gw],
                        Act.Tanh if j == 0 else Act.Sigmoid,
                    )
                t1 = wk.tile([128, gw], fp32, tag="t1")
                nc.vector.tensor_mul(t1[:], a[:, gw:2 * gw], a[:, 0:gw])  # i*g~
                c_new = cp.tile([128, gw], fp32, tag="c")
                if t == 0:
                    nc.vector.tensor_copy(c_new[:], t1[:])
                else:
                    nc.vector.tensor_mul(c_new[:], a[:, 2 * gw:3 * gw], c_prev[:])
                    nc.vector.tensor_add(c_new[:], c_new[:], t1[:])
                tch = wk.tile([128, gw], fp32, tag="tch")
                nc.scalar.activation(tch[:], c_new[:], Act.Tanh)
                nc.vector.tensor_mul(
                    h_all[:, :, (t + 1) * BL:(t + 2) * BL],
                    a[:, 3 * gw:4 * gw].rearrange("p (k b) -> p k b", b=BL),
                    tch[:].rearrange("p (k b) -> p k b", b=BL),
                )
                c_prev = c_new

            # ---- Phase 3: partial FC ----
            for m in range(NOUT // 128):
                for ch in range(nch):
                    ps = psb.tile([128, chunk], fp32, tag="big")
                    for k in range(KT):
                        nc.tensor.matmul(
                            ps[:],
                            fcw[:, k, m * 128:(m + 1) * 128],
                            h_all[:, k, BL + ch * chunk:BL + (ch + 1) * chunk],
                            start=(k == 0),
                            stop=(k == KT - 1),
                        )
                    st = stp.tile([128, chunk], fp32, tag="ost")
                    nc.vector.tensor_copy(st[:], ps[:])
                    nc.sync.dma_start(outT_d[m, :, ch * chunk:(ch + 1) * chunk], st[:])

    nc.compile()
    return nc


def _get_program(t_steps=T):
    if t_steps not in _CACHE:
        _CACHE[t_steps] = _build_program(t_steps)
    return _CACHE[t_steps]


def _to_bf16(arr):
    import ml_dtypes

    return np.asarray(arr).astype(ml_dtypes.bfloat16)


def _prep_weight_T(w_gate_rows):
    """[rows, 512] (gate-permuted rows) -> lhsT layout [128, KT, rows]."""
    wt = np.ascontiguousarray(w_gate_rows.T)  # [512, rows]
    return _to_bf16(wt.reshape(KT, 128, wt.shape[1]).transpose(1, 0, 2))


def _gate_perm_rows(w):
    blocks = np.split(np.asarray(w), 4, axis=0)
    return np.concatenate([blocks[i] for i in GATE_PERM], axis=0)


def _make_in_maps(x, w_ih_f, w_hh_f, b_ih_f, b_hh_f, w_ih_b, w_hh_b, b_ih_b,
                  b_hh_b, fc_w, fc_b, t_steps):
    per_dir = []
    for d, (wih, whh, bih, bhh) in enumerate(
        [(w_ih_f, w_hh_f, b_ih_f, b_hh_f), (w_ih_b, w_hh_b, b_ih_b, b_hh_b)]
    ):
        wih_r = _gate_perm_rows(wih)
        whh_r = _gate_perm_rows(whh)
        bias_r = _gate_perm_rows((np.asarray(bih) + np.asarray(bhh))[:, None])[:, 0]
        per_dir.append({
            "wihT": _prep_weight_T(wih_r),
            "whhT": _prep_weight_T(whh_r),
            "fcwT": _prep_weight_T(np.ascontiguousarray(
                np.asarray(fc_w)[:, d * H:(d + 1) * H])),
            "bias": np.ascontiguousarray(
                bias_r.reshape(MT, 128).T).astype(np.float32),
        })
    in_maps = []
    for c in range(8):
        d, q = c // 4, c % 4
        xq = np.asarray(x)[:t_steps, q * BL:(q + 1) * BL, :]
        if d == 1:
            xq = xq[::-1]
        xT = xq.transpose(2, 0, 1).reshape(KT, 128, t_steps * BL).transpose(1, 0, 2)
        m = dict(per_dir[d])
        m["xT"] = _to_bf16(xT)
        in_maps.append(m)
    return in_maps


def _assemble(results, fc_b, t_steps):
    out = np.zeros((t_steps, B, NOUT), np.float32)
    for c in range(8):
        d, q = c // 4, c % 4
        oT = np.asarray(results[c]["outT"]).reshape(NOUT, t_steps, BL)
        part = oT.transpose(1, 2, 0)  # [t, b, out]
        if d == 1:
            part = part[::-1]
        out[:, q * BL:(q + 1) * BL, :] += part
    out += np.asarray(fc_b, np.float32)
    return out


def kernel(x, w_ih_f, w_hh_f, b_ih_f, b_hh_f, w_ih_b, w_hh_b, b_ih_b, b_hh_b,
           fc_w, fc_b, _t_steps=T, _trace=False, _trace_kwargs=None):
    from concourse.bass_utils import run_bass_kernel_spmd

    nc = _get_program(_t_steps)
    in_maps = _make_in_maps(x, w_ih_f, w_hh_f, b_ih_f, b_hh_f, w_ih_b, w_hh_b,
                            b_ih_b, b_hh_b, fc_w, fc_b, _t_steps)
    res = run_bass_kernel_spmd(
        nc, in_maps, core_ids=list(range(8)), trace=_trace,
        **(_trace_kwargs or {}),
    )
    out = _assemble(res.results, fc_b, _t_steps)
    if _trace:
        kernel._last_result = res
    return out


# revision 6
# speedup vs baseline: 1.2333x; 1.2333x over previous
"""BiLSTM (T=256, B=64, NIN=H=NOUT=512) Trainium2 kernel over 8 NeuronCores.

Sharding: direction (2) x batch-quarter (4) = 8 cores, SPMD (one program).
Each core runs one direction's LSTM for 16 batch rows (backward cores get
time-reversed x), then computes its half of the final FC:
    out = h_f @ fc_w[:, :H].T + h_b @ fc_w[:, H:].T + fc_b
The host sums the two partial FC outputs per batch quarter. No collectives.

On-device layout (per core):
  - All matmuls "formulation G": out.T tiles [gate-rows on partitions, cols],
    lhsT = weight.T tiles (stationary), rhs = x.T / h.T tiles (moving).
  - Gate order reordered [g,i,f,o] so ACT work pipelines behind the PE.
  - h.T history kept fully resident in SBUF; xg staged through DRAM fp32.
"""

import numpy as np

T, B, NIN, H, NOUT = 256, 64, 512, 512, 512
BL = B // 4          # local batch per core (batch quarter)
KT = H // 128        # 4 k-tiles over the hidden/contraction dim
MT = (4 * H) // 128  # 16 m-tiles over the gate dim
# PyTorch gate blocks [i,f,g,o] -> our order [g,i,f,o]
GATE_PERM = [2, 0, 1, 3]

_CACHE = {}


def _build_program(t_steps):
    import concourse.mybir as mybir
    import concourse.tile as tile
    from concourse import bacc

    fp32 = mybir.dt.float32
    bf16 = mybir.dt.bfloat16
    Act = mybir.ActivationFunctionType

    ntb = t_steps * BL
    chunk = min(512, ntb)
    nch = ntb // chunk

    nc = bacc.Bacc("TRN2", target_bir_lowering=False, debug=False)
    xT_d = nc.dram_tensor("xT", [128, KT, ntb], bf16, kind="ExternalInput")
    wih_d = nc.dram_tensor("wihT", [128, KT, 4 * H], bf16, kind="ExternalInput")
    whh_d = nc.dram_tensor("whhT", [128, KT, 4 * H], bf16, kind="ExternalInput")
    fcw_d = nc.dram_tensor("fcwT", [128, KT, NOUT], bf16, kind="ExternalInput")
    bias_d = nc.dram_tensor("bias", [128, MT], fp32, kind="ExternalInput")
    outT_d = nc.dram_tensor("outT", [NOUT // 128, 128, ntb], fp32, kind="ExternalOutput")

    with tile.TileContext(nc) as tc:
        with (
            tc.tile_pool(name="weights", bufs=1) as wp,
            tc.tile_pool(name="state", bufs=1) as sp,
            tc.tile_pool(name="xgdram", bufs=1, space="DRAM") as dp,
            tc.tile_pool(name="stage", bufs=3) as stp,
            tc.tile_pool(name="work", bufs=2) as wk,
            tc.tile_pool(name="cpool", bufs=2) as cp,
            tc.tile_pool(name="xgin", bufs=3) as xgp,
            tc.tile_pool(name="psg", bufs=2, space="PSUM") as psg,
            tc.tile_pool(name="psb", bufs=2, space="PSUM") as psb,
        ):
            xT = wp.tile([128, KT, ntb], bf16)
            wih = wp.tile([128, KT, 4 * H], bf16)
            whh = wp.tile([128, KT, 4 * H], bf16)
            fcw = wp.tile([128, KT, NOUT], bf16)
            bias = wp.tile([128, MT], fp32)
            h_all = sp.tile([128, KT, (t_steps + 1) * BL], bf16)
            xg_dram = dp.tile([MT, 128, t_steps * BL], fp32)

            nc.sync.dma_start(xT[:], xT_d[:])
            nc.sync.dma_start(wih[:], wih_d[:])
            nc.sync.dma_start(whh[:], whh_d[:])
            nc.sync.dma_start(fcw[:], fcw_d[:])
            nc.sync.dma_start(bias[:], bias_d[:])
            nc.vector.memset(h_all[:, :, 0:BL], 0.0)

            # ---- Phase 1: xg = W_ih @ x.T + bias, staged to DRAM ----
            # m-major layout xg_dram[m, p, t*BL+b]: contiguous writes, and
            # per-16-step block reads are 1KB-contiguous per (p, m).
            for m in range(MT):
                for ch in range(nch):
                    ps = psb.tile([128, chunk], fp32, tag="big")
                    for k in range(KT):
                        nc.tensor.matmul(
                            ps[:],
                            wih[:, k, m * 128:(m + 1) * 128],
                            xT[:, k, ch * chunk:(ch + 1) * chunk],
                            start=(k == 0),
                            stop=(k == KT - 1),
                        )
                    st = stp.tile([128, chunk], fp32, tag="xgst")
                    nc.vector.tensor_scalar_add(st[:], ps[:], bias[:, m:m + 1])
                    nc.sync.dma_start(
                        xg_dram[m, :, ch * chunk:(ch + 1) * chunk], st[:])

            # ---- Phase 2: recurrence ----
            tb = min(16, t_steps)  # timesteps per xg block read
            nblk = t_steps // tb
            c_prev = None
            xgb = None
            for t in range(t_steps):
                if t % tb == 0:
                    xgb = xgp.tile([128, MT, tb * BL], fp32, tag="xgb")
                    src = xg_dram[:, :, t * BL:(t + tb) * BL].rearrange(
                        "m p c -> p m c")
                    nc.gpsimd.dma_start(xgb[:], src)
                tof = (t % tb) * BL  # column offset of step t inside the block

                gp = psg.tile([128, MT * BL], fp32, tag="gates")
                u = wk.tile([128, 4 * KT * BL], fp32, tag="u")
                a = wk.tile([128, 4 * KT * BL], fp32, tag="a")
                gw = KT * BL  # 64 columns per gate group
                for j in range(4):  # gate groups in order [g, i, f, o]
                    for mm in range(4):
                        m = j * 4 + mm
                        for k in range(KT):
                            nc.tensor.matmul(
                                gp[:, m * BL:(m + 1) * BL],
                                whh[:, k, m * 128:(m + 1) * 128],
                                h_all[:, k, t * BL:(t + 1) * BL],
                                start=(k == 0),
                                stop=(k == KT - 1),
                            )
                    nc.vector.tensor_add(
                        u[:, j * gw:(j + 1) * gw].rearrange(
                            "p (m b) -> p m b", b=BL),
                        gp[:, j * gw:(j + 1) * gw].rearrange(
                            "p (m b) -> p m b", b=BL),
                        xgb[:, j * 4:(j + 1) * 4, tof:tof + BL],
                    )
                    nc.scalar.activation(
                        a[:, j * gw:(j + 1) * gw],
                        u[:, j * gw:(j + 1) * gw],
                        Act.Tanh if j == 0 else Act.Sigmoid,
                    )
                t1 = wk.tile([128, gw], fp32, tag="t1")
                nc.vector.tensor_mul(t1[:], a[:, gw:2 * gw], a[:, 0:gw])  # i*g~
                c_new = cp.tile([128, gw], fp32, tag="c")
                if t == 0:
                    nc.vector.tensor_copy(c_new[:], t1[:])
                else:
                    nc.vector.tensor_mul(c_new[:], a[:, 2 * gw:3 * gw], c_prev[:])
                    nc.vector.tensor_add(c_new[:], c_new[:], t1[:])
                tch = wk.tile([128, gw], fp32, tag="tch")
                nc.scalar.activation(tch[:], c_new[:], Act.Tanh)
                nc.vector.tensor_mul(
                    h_all[:, :, (t + 1) * BL:(t + 2) * BL],
                    a[:, 3 * gw:4 * gw].rearrange("p (k b) -> p k b", b=BL),
                    tch[:].rearrange("p (k b) -> p k b", b=BL),
                )
                c_prev = c_new

            # ---- Phase 3: partial FC ----
            for m in range(NOUT // 128):
                for ch in range(nch):
                    ps = psb.tile([128, chunk], fp32, tag="big")
                    for k in range(KT):
                        nc.tensor.matmul(
                            ps[:],
                            fcw[:, k, m * 128:(m + 1) * 128],
                            h_all[:, k, BL + ch * chunk:BL + (ch + 1) * chunk],
                            start=(k == 0),
                            stop=(k == KT - 1),
                        )
                    st = stp.tile([128, chunk], fp32, tag="ost")
                    nc.vector.tensor_copy(st[:], ps[:])
                    nc.sync.dma_start(outT_d[m, :, ch * chunk:(ch + 1) * chunk], st[:])

    nc.compile()
    return nc


def _get_program(t_steps=T):
    if t_steps not in _CACHE:
        _CACHE[t_steps] = _build_program(t_steps)
    return _CACHE[t_steps]


def _to_bf16(arr):
    import ml_dtypes

    return np.asarray(arr).astype(ml_dtypes.bfloat16)


def _prep_weight_T(w_gate_rows):
    """[rows, 512] (gate-permuted rows) -> lhsT layout [128, KT, rows]."""
    wt = np.ascontiguousarray(w_gate_rows.T)  # [512, rows]
    return _to_bf16(wt.reshape(KT, 128, wt.shape[1]).transpose(1, 0, 2))


def _gate_perm_rows(w):
    blocks = np.split(np.asarray(w), 4, axis=0)
    return np.concatenate([blocks[i] for i in GATE_PERM], axis=0)


def _make_in_maps(x, w_ih_f, w_hh_f, b_ih_f, b_hh_f, w_ih_b, w_hh_b, b_ih_b,
                  b_hh_b, fc_w, fc_b, t_steps):
    per_dir = []
    for d, (wih, whh, bih, bhh) in enumerate(
        [(w_ih_f, w_hh_f, b_ih_f, b_hh_f), (w_ih_b, w_hh_b, b_ih_b, b_hh_b)]
    ):
        wih_r = _gate_perm_rows(wih)
        whh_r = _gate_perm_rows(whh)
        bias_r = _gate_perm_rows((np.asarray(bih) + np.asarray(bhh))[:, None])[:, 0]
        per_dir.append({
            "wihT": _prep_weight_T(wih_r),
            "whhT": _prep_weight_T(whh_r),
            "fcwT": _prep_weight_T(np.ascontiguousarray(
                np.asarray(fc_w)[:, d * H:(d + 1) * H])),
            "bias": np.ascontiguousarray(
                bias_r.reshape(MT, 128).T).astype(np.float32),
        })
    in_maps = []
    for c in range(8):
        d, q = c // 4, c % 4
        xq = np.asarray(x)[:t_steps, q * BL:(q + 1) * BL, :]
        if d == 1:
            xq = xq[::-1]
        xT = xq.transpose(2, 0, 1).reshape(KT, 128, t_steps * BL).transpose(1, 0, 2)
        m = dict(per_dir[d])
        m["xT"] = _to_bf16(xT)
        in_maps.append(m)
    return in_maps


def _assemble(results, fc_b, t_steps):
    out = np.zeros((t_steps, B, NOUT), np.float32)
    for c in range(8):
        d, q = c // 4, c % 4
        oT = np.asarray(results[c]["outT"]).reshape(NOUT, t_steps, BL)
        part = oT.transpose(1, 2, 0)  # [t, b, out]
        if d == 1:
            part = part[::-1]
        out[:, q * BL:(q + 1) * BL, :] += part
    out += np.asarray(fc_b, np.float32)
    return out


def kernel(x, w_ih_f, w_hh_f, b_ih_f, b_hh_f, w_ih_b, w_hh_b, b_ih_b, b_hh_b,
           fc_w, fc_b, _t_steps=T, _trace=False, _trace_kwargs=None):
    from concourse.bass_utils import run_bass_kernel_spmd

    nc = _get_program(_t_steps)
    in_maps = _make_in_maps(x, w_ih_f, w_hh_f, b_ih_f, b_hh_f, w_ih_b, w_hh_b,
                            b_ih_b, b_hh_b, fc_w, fc_b, _t_steps)
    res = run_bass_kernel_spmd(
        nc, in_maps, core_ids=list(range(8)), trace=_trace,
        **(_trace_kwargs or {}),
    )
    out = _assemble(res.results, fc_b, _t_steps)
    if _trace:
        kernel._last_result = res
    return out


# revision 8
# speedup vs baseline: 1.5473x; 1.2546x over previous
"""BiLSTM (T=256, B=64, NIN=H=NOUT=512) Trainium2 kernel over 8 NeuronCores.

Sharding: direction (2) x batch-quarter (4) = 8 cores, SPMD (one program).
Each core runs one direction's LSTM for 16 batch rows (backward cores get
time-reversed x), then computes its half of the final FC:
    out = h_f @ fc_w[:, :H].T + h_b @ fc_w[:, H:].T + fc_b
The host sums the two partial FC outputs per batch quarter. No collectives.

On-device layout (per core):
  - All matmuls "formulation G": out.T tiles [gate-rows on partitions, cols],
    lhsT = weight.T tiles (stationary), rhs = x.T / h.T tiles (moving).
  - Gate order reordered [g,i,f,o] so ACT work pipelines behind the PE.
  - h.T history kept fully resident in SBUF; xg staged through DRAM fp32.
"""

import numpy as np

T, B, NIN, H, NOUT = 256, 64, 512, 512, 512
BL = B // 4          # local batch per core (batch quarter)
KT = H // 128        # 4 k-tiles over the hidden/contraction dim
MT = (4 * H) // 128  # 16 m-tiles over the gate dim
# PyTorch gate blocks [i,f,g,o] -> our order [g,i,f,o]
GATE_PERM = [2, 0, 1, 3]

_CACHE = {}


def _build_program(t_steps):
    import concourse.mybir as mybir
    import concourse.tile as tile
    from concourse import bacc

    fp32 = mybir.dt.float32
    bf16 = mybir.dt.bfloat16
    Act = mybir.ActivationFunctionType

    ntb = t_steps * BL
    chunk = min(512, ntb)
    nch = ntb // chunk

    nc = bacc.Bacc("TRN2", target_bir_lowering=False, debug=False)
    xT_d = nc.dram_tensor("xT", [128, KT, ntb], bf16, kind="ExternalInput")
    wih_d = nc.dram_tensor("wihT", [128, KT, 4 * H], bf16, kind="ExternalInput")
    whh_d = nc.dram_tensor("whhT", [128, KT, 4 * H], bf16, kind="ExternalInput")
    fcw_d = nc.dram_tensor("fcwT", [128, KT, NOUT], bf16, kind="ExternalInput")
    bias_d = nc.dram_tensor("bias", [128, MT], fp32, kind="ExternalInput")
    outT_d = nc.dram_tensor("outT", [NOUT // 128, 128, ntb], fp32, kind="ExternalOutput")

    with tile.TileContext(nc) as tc:
        with (
            tc.tile_pool(name="weights", bufs=1) as wp,
            tc.tile_pool(name="state", bufs=1) as sp,
            tc.tile_pool(name="xgdram", bufs=1, space="DRAM") as dp,
            tc.tile_pool(name="stage", bufs=3) as stp,
            tc.tile_pool(name="work", bufs=2) as wk,
            tc.tile_pool(name="cpool", bufs=2) as cp,
            tc.tile_pool(name="xgin", bufs=3) as xgp,
            tc.tile_pool(name="psg", bufs=6, space="PSUM") as psg,
            tc.tile_pool(name="psb", bufs=2, space="PSUM") as psb,
        ):
            xT = wp.tile([128, KT, ntb], bf16)
            wih = wp.tile([128, KT, 4 * H], bf16)
            whh = wp.tile([128, KT, 4 * H], bf16)
            fcw = wp.tile([128, KT, NOUT], bf16)
            bias = wp.tile([128, MT], fp32)
            h_all = sp.tile([128, KT, (t_steps + 1) * BL], bf16)
            xg_dram = dp.tile([MT, 128, t_steps * BL], fp32)

            nc.sync.dma_start(xT[:], xT_d[:])
            nc.sync.dma_start(wih[:], wih_d[:])
            nc.sync.dma_start(whh[:], whh_d[:])
            nc.sync.dma_start(fcw[:], fcw_d[:])
            nc.sync.dma_start(bias[:], bias_d[:])
            nc.vector.memset(h_all[:, :, 0:BL], 0.0)

            # ---- Phase 1: xg = W_ih @ x.T + bias, staged to DRAM ----
            # m-major layout xg_dram[m, p, t*BL+b]: contiguous writes, and
            # per-16-step block reads are 1KB-contiguous per (p, m).
            for m in range(MT):
                for ch in range(nch):
                    ps = psb.tile([128, chunk], fp32, tag="big")
                    for k in range(KT):
                        nc.tensor.matmul(
                            ps[:],
                            wih[:, k, m * 128:(m + 1) * 128],
                            xT[:, k, ch * chunk:(ch + 1) * chunk],
                            start=(k == 0),
                            stop=(k == KT - 1),
                        )
                    st = stp.tile([128, chunk], fp32, tag="xgst")
                    nc.vector.tensor_scalar_add(st[:], ps[:], bias[:, m:m + 1])
                    nc.sync.dma_start(
                        xg_dram[m, :, ch * chunk:(ch + 1) * chunk], st[:])

            # ---- Phase 2: recurrence ----
            tb = min(16, t_steps)  # timesteps per xg block read
            nblk = t_steps // tb
            c_prev = None
            xgb = None
            for t in range(t_steps):
                if t % tb == 0:
                    xgb = xgp.tile([128, MT, tb * BL], fp32, tag="xgb")
                    src = xg_dram[:, :, t * BL:(t + tb) * BL].rearrange(
                        "m p c -> p m c")
                    nc.gpsimd.dma_start(xgb[:], src)
                tof = (t % tb) * BL  # column offset of step t inside the block

                u = wk.tile([128, 4 * KT * BL], fp32, tag="u")
                a = wk.tile([128, 4 * KT * BL], fp32, tag="a")
                gw = KT * BL  # 64 columns per gate group
                for j in range(4):  # gate groups in order [g, i, f, o]
                    # one PSUM bank per gate group so the DVE read of group j
                    # can overlap the PE writes of group j+1 (bank-safe)
                    gp = psg.tile([128, gw], fp32, tag="gates")
                    for mm in range(4):
                        m = j * 4 + mm
                        for k in range(KT):
                            nc.tensor.matmul(
                                gp[:, mm * BL:(mm + 1) * BL],
                                whh[:, k, m * 128:(m + 1) * 128],
                                h_all[:, k, t * BL:(t + 1) * BL],
                                start=(k == 0),
                                stop=(k == KT - 1),
                            )
                    nc.vector.tensor_add(
                        u[:, j * gw:(j + 1) * gw].rearrange(
                            "p (m b) -> p m b", b=BL),
                        gp[:].rearrange("p (m b) -> p m b", b=BL),
                        xgb[:, j * 4:(j + 1) * 4, tof:tof + BL],
                    )
                    nc.scalar.activation(
                        a[:, j * gw:(j + 1) * gw],
                        u[:, j * gw:(j + 1) * gw],
                        Act.Tanh if j == 0 else Act.Sigmoid,
                    )
                t1 = wk.tile([128, gw], fp32, tag="t1")
                nc.vector.tensor_mul(t1[:], a[:, gw:2 * gw], a[:, 0:gw])  # i*g~
                c_new = cp.tile([128, gw], fp32, tag="c")
                if t == 0:
                    nc.vector.tensor_copy(c_new[:], t1[:])
                else:
                    nc.vector.tensor_mul(c_new[:], a[:, 2 * gw:3 * gw], c_prev[:])
                    nc.vector.tensor_add(c_new[:], c_new[:], t1[:])
                tch = wk.tile([128, gw], fp32, tag="tch")
                nc.scalar.activation(tch[:], c_new[:], Act.Tanh)
                nc.vector.tensor_mul(
                    h_all[:, :, (t + 1) * BL:(t + 2) * BL],
                    a[:, 3 * gw:4 * gw].rearrange("p (k b) -> p k b", b=BL),
                    tch[:].rearrange("p (k b) -> p k b", b=BL),
                )
                c_prev = c_new

            # ---- Phase 3: partial FC ----
            for m in range(NOUT // 128):
                for ch in range(nch):
                    ps = psb.tile([128, chunk], fp32, tag="big")
                    for k in range(KT):
                        nc.tensor.matmul(
                            ps[:],
                            fcw[:, k, m * 128:(m + 1) * 128],
                            h_all[:, k, BL + ch * chunk:BL + (ch + 1) * chunk],
                            start=(k == 0),
                            stop=(k == KT - 1),
                        )
                    st = stp.tile([128, chunk], fp32, tag="ost")
                    nc.vector.tensor_copy(st[:], ps[:])
                    nc.sync.dma_start(outT_d[m, :, ch * chunk:(ch + 1) * chunk], st[:])

    nc.compile()
    return nc


def _get_program(t_steps=T):
    if t_steps not in _CACHE:
        _CACHE[t_steps] = _build_program(t_steps)
    return _CACHE[t_steps]


def _to_bf16(arr):
    import ml_dtypes

    return np.asarray(arr).astype(ml_dtypes.bfloat16)


def _prep_weight_T(w_gate_rows):
    """[rows, 512] (gate-permuted rows) -> lhsT layout [128, KT, rows]."""
    wt = np.ascontiguousarray(w_gate_rows.T)  # [512, rows]
    return _to_bf16(wt.reshape(KT, 128, wt.shape[1]).transpose(1, 0, 2))


def _gate_perm_rows(w):
    blocks = np.split(np.asarray(w), 4, axis=0)
    return np.concatenate([blocks[i] for i in GATE_PERM], axis=0)


def _make_in_maps(x, w_ih_f, w_hh_f, b_ih_f, b_hh_f, w_ih_b, w_hh_b, b_ih_b,
                  b_hh_b, fc_w, fc_b, t_steps):
    per_dir = []
    for d, (wih, whh, bih, bhh) in enumerate(
        [(w_ih_f, w_hh_f, b_ih_f, b_hh_f), (w_ih_b, w_hh_b, b_ih_b, b_hh_b)]
    ):
        wih_r = _gate_perm_rows(wih)
        whh_r = _gate_perm_rows(whh)
        bias_r = _gate_perm_rows((np.asarray(bih) + np.asarray(bhh))[:, None])[:, 0]
        per_dir.append({
            "wihT": _prep_weight_T(wih_r),
            "whhT": _prep_weight_T(whh_r),
            "fcwT": _prep_weight_T(np.ascontiguousarray(
                np.asarray(fc_w)[:, d * H:(d + 1) * H])),
            "bias": np.ascontiguousarray(
                bias_r.reshape(MT, 128).T).astype(np.float32),
        })
    in_maps = []
    for c in range(8):
        d, q = c // 4, c % 4
        xq = np.asarray(x)[:t_steps, q * BL:(q + 1) * BL, :]
        if d == 1:
            xq = xq[::-1]
        xT = xq.transpose(2, 0, 1).reshape(KT, 128, t_steps * BL).transpose(1, 0, 2)
        m = dict(per_dir[d])
        m["xT"] = _to_bf16(xT)
        in_maps.append(m)
    return in_maps


def _assemble(results, fc_b, t_steps):
    out = np.zeros((t_steps, B, NOUT), np.float32)
    for c in range(8):
        d, q = c // 4, c % 4
        oT = np.asarray(results[c]["outT"]).reshape(NOUT, t_steps, BL)
        part = oT.transpose(1, 2, 0)  # [t, b, out]
        if d == 1:
            part = part[::-1]
        out[:, q * BL:(q + 1) * BL, :] += part
    out += np.asarray(fc_b, np.float32)
    return out


def kernel(x, w_ih_f, w_hh_f, b_ih_f, b_hh_f, w_ih_b, w_hh_b, b_ih_b, b_hh_b,
           fc_w, fc_b, _t_steps=T, _trace=False, _trace_kwargs=None):
    from concourse.bass_utils import run_bass_kernel_spmd

    nc = _get_program(_t_steps)
    in_maps = _make_in_maps(x, w_ih_f, w_hh_f, b_ih_f, b_hh_f, w_ih_b, w_hh_b,
                            b_ih_b, b_hh_b, fc_w, fc_b, _t_steps)
    res = run_bass_kernel_spmd(
        nc, in_maps, core_ids=list(range(8)), trace=_trace,
        **(_trace_kwargs or {}),
    )
    out = _assemble(res.results, fc_b, _t_steps)
    if _trace:
        kernel._last_result = res
    return out


# revision 11
# speedup vs baseline: 1.8905x; 1.2218x over previous
"""BiLSTM (T=256, B=64, NIN=H=NOUT=512) Trainium2 kernel over 8 NeuronCores.

Sharding: direction (2) x batch-quarter (4) = 8 cores, SPMD (one program).
Each core runs one direction's LSTM for 16 batch rows (backward cores get
time-reversed x), then computes its half of the final FC:
    out = h_f @ fc_w[:, :H].T + h_b @ fc_w[:, H:].T + fc_b
The host sums the two partial FC outputs per batch quarter. No collectives.

Single fused device loop:
  - Recurrence matmuls in "formulation G": gates.T tiles on partitions,
    lhsT = W_hh.T tiles (stationary), rhs = h.T tiles (16 batch cols moving).
  - xg = W_ih@x.T+b precompute units (big-N matmuls) interleaved 2 chunks
    ahead of consumption; xg ring lives in SBUF bf16 (no DRAM roundtrip).
  - xg is added into each gate-group's PSUM bank by an identity matmul
    emitted FIRST in the accumulation group (start=True), so ScalarE applies
    sigmoid/tanh directly from PSUM - no DVE pre-add on the critical chain.
  - One PSUM bank per gate group so ACT/DVE reads overlap PE writes.
  - FC output units interleaved once the needed h chunk is complete.
  - Gate order [f, i, g, o] to start the c-chain as early as possible.
"""

import numpy as np

T, B, NIN, H, NOUT = 256, 64, 512, 512, 512
BL = B // 4          # local batch per core (batch quarter)
KT = H // 128        # 4 k-tiles over the hidden/contraction dim
MT = (4 * H) // 128  # 16 m-tiles over the gate dim
# PyTorch gate blocks [i,f,g,o] -> our order [f,i,g,o]
GATE_PERM = [1, 0, 2, 3]

_CACHE = {}


def _build_program(t_steps):
    import concourse.mybir as mybir
    import concourse.tile as tile
    from concourse import bacc
    from concourse.masks import make_identity

    fp32 = mybir.dt.float32
    bf16 = mybir.dt.bfloat16
    Act = mybir.ActivationFunctionType

    ntb = t_steps * BL
    chunk = min(512, ntb)
    nch = ntb // chunk
    spc = chunk // BL   # steps per chunk
    lead = min(2, nch)  # xg chunks computed ahead

    nc = bacc.Bacc("TRN2", target_bir_lowering=False, debug=False)
    xT_d = nc.dram_tensor("xT", [128, KT, ntb], bf16, kind="ExternalInput")
    wih_d = nc.dram_tensor("wihT", [128, KT, 4 * H], bf16, kind="ExternalInput")
    whh_d = nc.dram_tensor("whhT", [128, KT, 4 * H], bf16, kind="ExternalInput")
    fcw_d = nc.dram_tensor("fcwT", [128, KT, NOUT], bf16, kind="ExternalInput")
    bias_d = nc.dram_tensor("bias", [128, MT], fp32, kind="ExternalInput")
    outT_d = nc.dram_tensor("outT", [NOUT // 128, 128, ntb], fp32, kind="ExternalOutput")

    with tile.TileContext(nc) as tc:
        with (
            tc.tile_pool(name="weights", bufs=1) as wp,
            tc.tile_pool(name="state", bufs=1) as sp,
            tc.tile_pool(name="ring", bufs=lead + 1) as rp,
            tc.tile_pool(name="stage", bufs=3) as stp,
            tc.tile_pool(name="work", bufs=2) as wk,
            tc.tile_pool(name="cpool", bufs=2) as cp,
            tc.tile_pool(name="psg", bufs=6, space="PSUM") as psg,
            tc.tile_pool(name="psb", bufs=2, space="PSUM") as psb,
        ):
            xT = wp.tile([128, KT, ntb], bf16)
            wih = wp.tile([128, KT, 4 * H], bf16)
            whh = wp.tile([128, KT, 4 * H], bf16)
            fcw = wp.tile([128, KT, NOUT], bf16)
            bias = wp.tile([128, MT], fp32)
            ident = wp.tile([128, 128], bf16)
            h_all = sp.tile([128, KT, (t_steps + 1) * BL], bf16)

            nc.sync.dma_start(xT[:], xT_d[:])
            nc.sync.dma_start(wih[:], wih_d[:])
            nc.sync.dma_start(whh[:], whh_d[:])
            nc.sync.dma_start(fcw[:], fcw_d[:])
            nc.sync.dma_start(bias[:], bias_d[:])
            make_identity(nc, ident[:])
            nc.vector.memset(h_all[:, :, 0:BL], 0.0)

            rings = {}

            def xg_unit(ch, m):
                """xg.T chunk for m-tile m -> ring[ch] slice (bf16, +bias)."""
                if ch not in rings:
                    rings[ch] = rp.tile([128, MT, chunk], bf16, tag="ring",
                                        name=f"ring{ch}")
                ps = psb.tile([128, chunk], fp32, tag="big")
                for k in range(KT):
                    nc.tensor.matmul(
                        ps[:], wih[:, k, m * 128:(m + 1) * 128],
                        xT[:, k, ch * chunk:(ch + 1) * chunk],
                        start=(k == 0), stop=(k == KT - 1))
                nc.vector.tensor_scalar_add(rings[ch][:, m, :], ps[:],
                                            bias[:, m:m + 1])

            def fc_unit(ch, m):
                ps = psb.tile([128, chunk], fp32, tag="big")
                for k in range(KT):
                    nc.tensor.matmul(
                        ps[:], fcw[:, k, m * 128:(m + 1) * 128],
                        h_all[:, k, BL + ch * chunk:BL + (ch + 1) * chunk],
                        start=(k == 0), stop=(k == KT - 1))
                st = stp.tile([128, chunk], fp32, tag="ost")
                nc.vector.tensor_copy(st[:], ps[:])
                nc.sync.dma_start(outT_d[m, :, ch * chunk:(ch + 1) * chunk], st[:])

            # prologue: first `lead` xg chunks
            for ch in range(lead):
                for m in range(MT):
                    xg_unit(ch, m)

            n_fc = (NOUT // 128) * nch
            fc_done = 0
            gw = KT * BL  # 64 columns per gate group
            c_prev = None
            for t in range(t_steps):
                s = t % spc
                ch = t // spc
                ring = rings[ch]

                # interleave xg production for chunk ch+lead
                if ch + lead < nch:
                    for m in range(s * MT // spc, (s + 1) * MT // spc):
                        xg_unit(ch + lead, m)
                # interleave FC once its h chunk is fully written;
                # spread at most 1 unit per 8 steps
                want_fc = 0
                if t >= spc:
                    want_fc = min(n_fc, 4 * (t // spc), (t - spc) // 8 + 1)
                while fc_done < want_fc:
                    fc_unit(fc_done // (NOUT // 128), fc_done % (NOUT // 128))
                    fc_done += 1

                a = wk.tile([128, 4 * gw], fp32, tag="a")
                for j in range(4):  # gate groups in order [f, i, g, o]
                    gp = psg.tile([128, gw], fp32, tag="gates")
                    # identity matmul seeds the bank with xg (+ sets
                    # has_written for the whole tile), k-MMs accumulate
                    nc.tensor.matmul(
                        gp[:], ident[:],
                        ring[:, j * 4:(j + 1) * 4, s * BL:(s + 1) * BL],
                        start=True, stop=False)
                    for mm in range(4):
                        m = j * 4 + mm
                        for k in range(KT):
                            nc.tensor.matmul(
                                gp[:, mm * BL:(mm + 1) * BL],
                                whh[:, k, m * 128:(m + 1) * 128],
                                h_all[:, k, t * BL:(t + 1) * BL],
                                start=False,
                                stop=(mm == 3 and k == KT - 1))
                    nc.scalar.activation(
                        a[:, j * gw:(j + 1) * gw], gp[:],
                        Act.Tanh if j == 2 else Act.Sigmoid)
                    if j == 0 and t > 0:
                        c1 = wk.tile([128, gw], fp32, tag="c1")
                        nc.vector.tensor_mul(c1[:], a[:, 0:gw], c_prev[:])
                t1 = wk.tile([128, gw], fp32, tag="t1")
                nc.vector.tensor_mul(t1[:], a[:, gw:2 * gw], a[:, 2 * gw:3 * gw])
                c_new = cp.tile([128, gw], fp32, tag="c")
                if t == 0:
                    nc.vector.tensor_copy(c_new[:], t1[:])
                else:
                    nc.vector.tensor_add(c_new[:], c1[:], t1[:])
                tch = wk.tile([128, gw], fp32, tag="tch")
                nc.scalar.activation(tch[:], c_new[:], Act.Tanh)
                nc.vector.tensor_mul(
                    h_all[:, :, (t + 1) * BL:(t + 2) * BL],
                    a[:, 3 * gw:4 * gw].rearrange("p (k b) -> p k b", b=BL),
                    tch[:].rearrange("p (k b) -> p k b", b=BL))
                c_prev = c_new
                if ch - 1 in rings and s == spc - 1:
                    del rings[ch - 1]

            while fc_done < n_fc:  # FC epilogue
                fc_unit(fc_done // (NOUT // 128), fc_done % (NOUT // 128))
                fc_done += 1

    nc.compile()
    return nc


def _get_program(t_steps=T):
    if t_steps not in _CACHE:
        _CACHE[t_steps] = _build_program(t_steps)
    return _CACHE[t_steps]


def _to_bf16(arr):
    import ml_dtypes

    return np.asarray(arr).astype(ml_dtypes.bfloat16)


def _prep_weight_T(w_gate_rows):
    """[rows, 512] (gate-permuted rows) -> lhsT layout [128, KT, rows]."""
    wt = np.ascontiguousarray(w_gate_rows.T)  # [512, rows]
    return _to_bf16(wt.reshape(KT, 128, wt.shape[1]).transpose(1, 0, 2))


def _gate_perm_rows(w):
    blocks = np.split(np.asarray(w), 4, axis=0)
    return np.concatenate([blocks[i] for i in GATE_PERM], axis=0)


def _make_in_maps(x, w_ih_f, w_hh_f, b_ih_f, b_hh_f, w_ih_b, w_hh_b, b_ih_b,
                  b_hh_b, fc_w, fc_b, t_steps):
    per_dir = []
    for d, (wih, whh, bih, bhh) in enumerate(
        [(w_ih_f, w_hh_f, b_ih_f, b_hh_f), (w_ih_b, w_hh_b, b_ih_b, b_hh_b)]
    ):
        wih_r = _gate_perm_rows(wih)
        whh_r = _gate_perm_rows(whh)
        bias_r = _gate_perm_rows((np.asarray(bih) + np.asarray(bhh))[:, None])[:, 0]
        per_dir.append({
            "wihT": _prep_weight_T(wih_r),
            "whhT": _prep_weight_T(whh_r),
            "fcwT": _prep_weight_T(np.ascontiguousarray(
                np.asarray(fc_w)[:, d * H:(d + 1) * H])),
            "bias": np.ascontiguousarray(
                bias_r.reshape(MT, 128).T).astype(np.float32),
        })
    in_maps = []
    for c in range(8):
        d, q = c // 4, c % 4
        xq = np.asarray(x)[:t_steps, q * BL:(q + 1) * BL, :]
        if d == 1:
            xq = xq[::-1]
        xT = xq.transpose(2, 0, 1).reshape(KT, 128, t_steps * BL).transpose(1, 0, 2)
        m = dict(per_dir[d])
        m["xT"] = _to_bf16(xT)
        in_maps.append(m)
    return in_maps


def _assemble(results, fc_b, t_steps):
    out = np.zeros((t_steps, B, NOUT), np.float32)
    for c in range(8):
        d, q = c // 4, c % 4
        oT = np.asarray(results[c]["outT"]).reshape(NOUT, t_steps, BL)
        part = oT.transpose(1, 2, 0)  # [t, b, out]
        if d == 1:
            part = part[::-1]
        out[:, q * BL:(q + 1) * BL, :] += part
    out += np.asarray(fc_b, np.float32)
    return out


def kernel(x, w_ih_f, w_hh_f, b_ih_f, b_hh_f, w_ih_b, w_hh_b, b_ih_b, b_hh_b,
           fc_w, fc_b, _t_steps=T, _trace=False, _trace_kwargs=None):
    from concourse.bass_utils import run_bass_kernel_spmd

    nc = _get_program(_t_steps)
    in_maps = _make_in_maps(x, w_ih_f, w_hh_f, b_ih_f, b_hh_f, w_ih_b, w_hh_b,
                            b_ih_b, b_hh_b, fc_w, fc_b, _t_steps)
    res = run_bass_kernel_spmd(
        nc, in_maps, core_ids=list(range(8)), trace=_trace,
        **(_trace_kwargs or {}),
    )
    out = _assemble(res.results, fc_b, _t_steps)
    if _trace:
        kernel._last_result = res
    return out
